# revision 68
# baseline (speedup 1.0000x reference)
"""Llama GQA attention (B=1, S=2048, HID=4096, 32 Q heads / 8 KV heads, RoPE,
causal) on 8 trn2 NeuronCores, tensor-parallel over KV heads.

Per core c: q-heads 4c..4c+3, kv-head c. Device computes a partial
out_c = attn_heads_c @ Wo[:, cols_c].T ; host sums the 8 partials.

Design (cost-model driven; PE floor ~331us, achieved ~343us):
  - single software-pipelined loop over q-blocks qb (512 rows each);
    emission interleaves attn(qb) chunk steps with proj(qb+1) and
    wout(qb-1) steps so the in-order PE stream never waits on the
    ACT exp latency
  - all weights resident in SBUF, loaded once with >=1KB contiguous
    runs (sub-512B DMA descriptors pay 2x), interleaved with the first
    x block; qb0 projection is ci-major so the PE starts as DMA lands
  - RoPE rotate-half via SBUF->SBUF DMA partition swap (sign folded
    into the sin table on host); V produced directly in [s,d]
    orientation from the projection (no PE transpose)
  - causal masking at 128-col granularity inside the diagonal 512
    block (scores/exp/PV region-restricted)
  - softmax row-sum: DVE chunk accumulation + GPSIMD
    partition_all_reduce + DVE reciprocal (no PE, no DRAM roundtrip)
  - bf16 partial output, two half-DMAs per 128-row chunk (per-eb for
    the last chunk to shorten the drain tail)
All matmuls bf16 with fp32 PSUM accumulation; 8 PSUM banks statically
split across 4 pools (proj/scores/PV/Wo).
"""
import math

import numpy as np
import ml_dtypes

S = 2048
HID = 4096
D = 128
NQ = 4            # q heads per core
NCORES = 8
SB = 512          # s/q block
NSB = S // SB     # 4
NKC = S // D      # 16 k chunks
NEB = HID // 512  # 8 output e blocks
NCC = HID // D    # 32 contraction chunks
SCALE = 1.0 / math.sqrt(D)
ROPE_THETA = 10000.0

BF16 = ml_dtypes.bfloat16

_CACHE = {}
FINE = True


def _build():
    import concourse.tile as tile
    from concourse import bacc, mybir

    dt = mybir.dt
    nc = bacc.Bacc("TRN2", target_bir_lowering=False, debug=False,
                   num_devices=NCORES)

    xT = nc.dram_tensor("xT", [HID, S], dt.bfloat16, kind="ExternalInput")
    wqT = nc.dram_tensor("wqT", [HID, NQ * D], dt.bfloat16, kind="ExternalInput")
    wkp = nc.dram_tensor("wkp", [D, HID], dt.bfloat16, kind="ExternalInput")
    wvp = nc.dram_tensor("wvp", [D, HID], dt.bfloat16, kind="ExternalInput")
    woT = nc.dram_tensor("woT", [NQ * D, HID], dt.bfloat16, kind="ExternalInput")
    cosT = nc.dram_tensor("cosT", [D, S], dt.bfloat16, kind="ExternalInput")
    sinT = nc.dram_tensor("sinT", [D, S], dt.bfloat16, kind="ExternalInput")
    maskd = nc.dram_tensor("maskd", [D, 4, SB], dt.bfloat16, kind="ExternalInput")
    part = nc.dram_tensor("part", [S, HID], dt.bfloat16, kind="ExternalOutput")

    xr = xT.rearrange("(ko p) s -> p ko s", p=D)                  # [128,32,2048]
    wqr = wqT.rearrange("(ko p) hd -> p ko hd", p=D)              # [128,32,512]
    wkr = wkp.rearrange("p (ko d) -> p ko d", d=D)                # [128,32,128]
    wvr = wvp.rearrange("p (ko d) -> p ko d", d=D)
    wor = woT.rearrange("(h p) (eb e) -> p h eb e", p=D, e=512)   # [128,4,8,512]

    from concourse import bass_isa

    with tile.TileContext(nc) as tc:
        _body(nc, tc, tile, mybir, bass_isa,
              xr, wqr, wkr, wvr, wor, cosT, sinT, maskd, part)
    nc.compile()
    return nc


def _body(nc, tc, tile, mybir, bass_isa,
          xr, wqr, wkr, wvr, wor, cosT, sinT, maskd, part):
    from contextlib import ExitStack

    dt = mybir.dt
    AF = mybir.ActivationFunctionType

    with ExitStack() as ctx:
        const = ctx.enter_context(tc.tile_pool(name="const", bufs=1))
        wper = ctx.enter_context(tc.tile_pool(name="wper", bufs=1))
        kvper = ctx.enter_context(tc.tile_pool(name="kvper", bufs=1))
        xpool = ctx.enter_context(tc.tile_pool(name="xpool", bufs=3))  # xp tag bufs=8
        qpool = ctx.enter_context(tc.tile_pool(name="qpool", bufs=2))
        epool = ctx.enter_context(tc.tile_pool(name="epool", bufs=7))
        espool = ctx.enter_context(tc.tile_pool(name="espool", bufs=2))
        atpool = ctx.enter_context(tc.tile_pool(name="atpool", bufs=2))
        opool = ctx.enter_context(tc.tile_pool(name="opool", bufs=2))
        rpool = ctx.enter_context(tc.tile_pool(name="rpool", bufs=2))
        # PSUM: 8 banks total, statically budgeted
        ps_a = ctx.enter_context(tc.tile_pool(name="ps_a", bufs=2, space="PSUM"))
        ps_b = ctx.enter_context(tc.tile_pool(name="ps_b", bufs=2, space="PSUM"))
        ps_c = ctx.enter_context(tc.tile_pool(name="ps_c", bufs=2, space="PSUM"))
        ps_w = ctx.enter_context(tc.tile_pool(name="ps_w", bufs=2, space="PSUM"))

        # ---- weights / constants (loaded once, interleaved with first x) ----
        wq = wper.tile([D, NCC, NQ * D], dt.bfloat16)   # 4.2 MB
        wk = wper.tile([D, NCC, D], dt.bfloat16)        # 1 MB
        wv = wper.tile([D, NCC, D], dt.bfloat16)
        wo = wper.tile([D, NQ, NEB, 512], dt.bfloat16)  # 4.2 MB
        cos = wper.tile([D, S], dt.bfloat16)
        sin = wper.tile([D, S], dt.bfloat16)
        mask = const.tile([D, 4, SB], dt.bfloat16)

        xps = {}

        def load_x(qb):
            """Four quarter tiles per q block for fine DMA/compute pipelining."""
            ssl = slice(qb * SB, (qb + 1) * SB)
            qs = []
            for qtr in range(4):
                t = xpool.tile([D, 8, SB], dt.bfloat16, tag="xp", bufs=8,
                               name=f"xp{qb}_{qtr}")
                nc.sync.dma_start(out=t, in_=xr[:, qtr * 8:(qtr + 1) * 8, ssl])
                qs.append(t)
            xps[qb] = qs

        def xchunk(qb, ci):
            return xps[qb][ci // 8][:, ci % 8]

        # startup order: interleave wk/wq pieces with x quarters so the
        # qb0 ci-major projection starts as soon as possible
        xps[0] = []

        def xq(qtr):
            t = xpool.tile([D, 8, SB], dt.bfloat16, tag="xp", bufs=8,
                           name=f"xp0_{qtr}")
            nc.sync.dma_start(out=t, in_=xr[:, qtr * 8:(qtr + 1) * 8, 0:SB])
            xps[0].append(t)

        nc.sync.dma_start(out=wk[:, 0:4], in_=wkr[:, 0:4])
        t0 = xpool.tile([D, 8, SB], dt.bfloat16, tag="xp", bufs=8, name="xp0_0")
        nc.sync.dma_start(out=t0[:, 0:4], in_=xr[:, 0:4, 0:SB])
        xps[0].append(t0)
        nc.sync.dma_start(out=wk[:, 4:8], in_=wkr[:, 4:8])
        nc.sync.dma_start(out=t0[:, 4:8], in_=xr[:, 4:8, 0:SB])
        nc.sync.dma_start(out=wq[:, 0:4], in_=wqr[:, 0:4])
        nc.sync.dma_start(out=wq[:, 4:8], in_=wqr[:, 4:8])
        xq(1)
        nc.sync.dma_start(out=wq[:, 8:16], in_=wqr[:, 8:16])
        nc.sync.dma_start(out=wk[:, 8:32], in_=wkr[:, 8:32])
        nc.sync.dma_start(out=wv, in_=wvr)
        xq(2)
        nc.sync.dma_start(out=wq[:, 16:24], in_=wqr[:, 16:24])
        xq(3)
        nc.sync.dma_start(out=wq[:, 24:32], in_=wqr[:, 24:32])
        nc.sync.dma_start(out=cos, in_=cosT[:, :])
        nc.sync.dma_start(out=sin, in_=sinT[:, :])
        nc.sync.dma_start(out=mask, in_=maskd[:, :, :])
        load_x(1)
        # NOTE: wo is loaded later (filler of qb0) so it does not delay xp(2)

        KT = kvper.tile([D, S], dt.bfloat16)            # [d, k]
        V = kvper.tile([D, NKC, D], dt.bfloat16)        # [k%, kc, d]

        def rope(acc_ps, out_slice, ssl, tagp):
            """out = raw*cos + halfswap(raw)*sin ; sin sign-folded on host.
            The half-swap runs on the (idle) DMA engines, not the PE."""
            raw = rpool.tile([D, SB], dt.bfloat16, tag="raw")
            nc.scalar.copy(out=raw, in_=acc_ps)
            sw = rpool.tile([D, SB], dt.bfloat16, tag="sw")
            nc.sync.dma_start(out=sw[0:64, :], in_=raw[64:128, :])
            nc.sync.dma_start(out=sw[64:128, :], in_=raw[0:64, :])
            nc.vector.tensor_mul(out=out_slice, in0=raw, in1=cos[:, ssl])
            b = rpool.tile([D, SB], dt.bfloat16, tag="b")
            nc.vector.tensor_mul(out=b, in0=sw, in1=sin[:, ssl])
            nc.vector.tensor_add(out=out_slice, in0=out_slice, in1=b)

        def proj0():
            """qb=0 projection, ci-major so the PE starts as DMA pieces land.
            Uses 6 concurrent accumulators borrowed across psum pools."""
            ssl = slice(0, SB)
            QTq = qpool.tile([D, NQ, SB], dt.bfloat16, tag="qt", name="qt0")
            kps = ps_c.tile([D, SB], dt.float32, tag="ps")
            qps = [ps_a.tile([D, SB], dt.float32, tag="ps", name=f"q0ps{h}")
                   for h in range(2)]
            qps += [ps_b.tile([D, SB], dt.float32, tag="ps", name=f"q0ps{h}")
                    for h in range(2, NQ)]
            vps = ps_w.tile([D, 4, D], dt.float32, tag="ps")
            for ci in range(NCC):
                st, sp = (ci == 0), (ci == NCC - 1)
                nc.tensor.matmul(kps, wk[:, ci], xchunk(0, ci),
                                 start=st, stop=sp)
                for h in range(NQ):
                    nc.tensor.matmul(qps[h], wq[:, ci, h * D:(h + 1) * D],
                                     xchunk(0, ci), start=st, stop=sp)
            for j in range(4):
                for ci in range(NCC):
                    nc.tensor.matmul(
                        vps[:, j], xchunk(0, ci)[:, j * D:(j + 1) * D],
                        wv[:, ci], start=(ci == 0), stop=(ci == NCC - 1))
            # q2/q3 first: each frees its borrowed ps_b slot (via its ACT
            # copy) before its own rotation matmul reallocates it
            for h in (2, 3):
                rope(qps[h], QTq[:, h], ssl, "q")
            rope(kps, KT[:, ssl], ssl, "k")
            for h in (0, 1):
                rope(qps[h], QTq[:, h], ssl, "q")
            for j in range(4):
                nc.vector.tensor_copy(out=V[:, j], in_=vps[:, j])
            return QTq

        def proj_gen(qb, sink):
            """QKV projections for q block qb (tgt-major, generator steps)."""
            ssl = slice(qb * SB, (qb + 1) * SB)
            QTq = qpool.tile([D, NQ, SB], dt.bfloat16, tag="qt", name=f"qt{qb}")
            sink[qb] = QTq
            for h in range(NQ):
                qps = ps_a.tile([D, SB], dt.float32, tag="ps")
                for ci in range(NCC):
                    nc.tensor.matmul(qps, wq[:, ci, h * D:(h + 1) * D],
                                     xchunk(qb, ci),
                                     start=(ci == 0), stop=(ci == NCC - 1))
                    if ci % 4 == 3:
                        yield
                rope(qps, QTq[:, h], ssl, "q")
                yield
            kps = ps_a.tile([D, SB], dt.float32, tag="ps")
            for ci in range(NCC):
                nc.tensor.matmul(kps, wk[:, ci], xchunk(qb, ci),
                                 start=(ci == 0), stop=(ci == NCC - 1))
                if ci % 4 == 3:
                    yield
            rope(kps, KT[:, ssl], ssl, "k")
            yield
            vps = ps_w.tile([D, 4, D], dt.float32, tag="ps")
            for j in range(4):
                for ci in range(NCC):
                    nc.tensor.matmul(
                        vps[:, j], xchunk(qb, ci)[:, j * D:(j + 1) * D],
                        wv[:, ci], start=(ci == 0), stop=(ci == NCC - 1))
                    if ci % 16 == 15:
                        yield
            for j in range(4):
                nc.vector.tensor_copy(out=V[:, 4 * qb + j], in_=vps[:, j])
            yield

        def attn_gen(qb, QTq, attnq, stp_pools=None):
            """Chunk-pipelined attention for q block qb -> attnq tile.

            Per head: scores+exp per k chunk (region-restricted on the causal
            diagonal), PV lags 2 chunks; the last two PV chunks plus the
            rowsum/normalize chain (on Pool+DVE, no PE) are deferred into the
            next head's score stream.
            """
            nkc = 4 * (qb + 1)
            sps = stp_pools or [ps_b]
            spi = [0]
            pend = []

            def head(h):
                esum = espool.tile([D, SB], dt.bfloat16, tag="es")
                pvp = ps_c.tile([D, SB], dt.float32, tag="ps")
                Ec = {}

                def pv(kc):
                    j = kc - 4 * qb
                    off = (j * D if j >= 1 else 0) if FINE else 0
                    nc.tensor.matmul(pvp[:, off:], V[:, kc], Ec.pop(kc)[:, off:],
                                     start=(kc == 0), stop=(kc == nkc - 1))

                def rowsum():
                    asum = rpool.tile([D, SB], dt.bfloat16, tag="as")
                    nc.gpsimd.partition_all_reduce(asum, esum, D,
                                                   bass_isa.ReduceOp.add)
                    rcpf = rpool.tile([D, SB], dt.bfloat16, tag="rcp")
                    with nc.allow_low_precision(
                            reason="bf16 softmax normalizer is within tol"):
                        nc.vector.reciprocal(out=rcpf, in_=asum)
                    nc.vector.tensor_mul(out=attnq[:, h], in0=pvp, in1=rcpf)

                for kc in range(nkc):
                    j = kc - 4 * qb
                    off = (j * D if j >= 1 else 0) if FINE else 0
                    E = epool.tile([D, SB], dt.bfloat16, tag="E",
                                   name=f"E{qb}_{h}_{kc}")
                    stp = sps[spi[0] % len(sps)].tile([D, SB], dt.float32,
                                                      tag="ps")
                    spi[0] += 1
                    nc.tensor.matmul(stp[:, off:], KT[:, kc * D:(kc + 1) * D],
                                     QTq[:, h, off:], start=True, stop=True)
                    nc.scalar.activation(out=E[:, off:], in_=stp[:, off:],
                                         func=AF.Exp, scale=SCALE)
                    if j >= 0:  # diagonal block: binary causal mask
                        nc.vector.tensor_mul(out=E[:, off:], in0=E[:, off:],
                                             in1=mask[:, j, off:])
                    if kc == 0:
                        nc.vector.tensor_copy(out=esum, in_=E)
                    else:
                        nc.vector.tensor_add(out=esum[:, off:],
                                             in0=esum[:, off:], in1=E[:, off:])
                    Ec[kc] = E
                    if pend and kc >= 2:
                        pend.pop(0)()
                        if pend and kc >= 3:
                            pend.pop(0)()
                    if kc >= 2 and kc - 2 < 4 * qb:
                        pv(kc - 2)   # off-diagonal: short lag is safe
                    yield
                # diagonal-block PVs wait on the exp->mask DVE chain; defer
                # them into the next head's score stream
                pend.extend([(lambda k: lambda: pv(k))(kc)
                             for kc in range(4 * qb, nkc)]
                            + [rowsum])

            for h in range(NQ):
                yield from head(h)
            for f in pend:
                f()

        def wout_gen(qb, attnq, pools=None):
            """Partial output projection for s rows of q block qb."""
            pools = pools or [ps_w, ps_a]
            gi = 0
            for sc in range(4):
                scl = slice(sc * D, (sc + 1) * D)
                for eb in range(NEB):
                    if eb % 4 == 0:
                        ost = opool.tile([D, 4, 512], dt.bfloat16, tag="o")
                    op = pools[gi % len(pools)].tile([D, 512], dt.float32,
                                                     tag="ps")
                    gi += 1
                    for h in range(NQ):
                        nc.tensor.matmul(op, attnq[:, h, scl], wo[:, h, eb],
                                         start=(h == 0), stop=(h == NQ - 1))
                    if eb % 2 == 0:
                        nc.scalar.copy(out=ost[:, eb % 4], in_=op)
                    else:
                        nc.vector.tensor_copy(out=ost[:, eb % 4], in_=op)
                    rsl = slice(qb * SB + sc * D, qb * SB + (sc + 1) * D)
                    if qb == NSB - 1 and sc == 3:
                        # last row chunk: per-eb DMAs to shorten the drain tail
                        nc.sync.dma_start(
                            out=part[rsl, eb * 512:(eb + 1) * 512],
                            in_=ost[:, eb % 4])
                    elif eb % 4 == 3:
                        half = eb // 4
                        nc.sync.dma_start(
                            out=part[rsl, half * 2048:(half + 1) * 2048],
                            in_=ost)
                    yield

        def filler_gen(qb, sink, attnqs):
            if qb + 2 < NSB:
                load_x(qb + 2)
            if qb == 0:
                nc.sync.dma_start(out=wo, in_=wor)
            if qb + 1 < NSB:
                yield from proj_gen(qb + 1, sink)
            if qb >= 1:
                yield from wout_gen(qb - 1, attnqs[qb - 1])

        QTqs = {0: proj0()}
        attnqs = {}
        for qb in range(NSB):
            attnqs[qb] = atpool.tile([D, NQ, SB], dt.bfloat16, tag="at",
                                     name=f"at{qb}")
            a = attn_gen(qb, QTqs[qb], attnqs[qb],
                         stp_pools=[ps_b, ps_a] if qb == NSB - 1 else None)
            f = filler_gen(qb, QTqs, attnqs)
            n_a = 4 * 4 * (qb + 1)            # attn chunk steps
            n_f = (32 if qb >= 1 else 0) + (54 if qb + 1 < NSB else 0)
            ratio = n_f / n_a
            acc = 0.0
            for _ in range(n_a):
                next(a, None)
                acc += ratio
                while acc >= 1.0:
                    next(f, None)
                    acc -= 1.0
            for _ in a:
                pass
            for _ in f:
                pass
        for _ in wout_gen(NSB - 1, attnqs[NSB - 1],
                          pools=[ps_w, ps_a, ps_b, ps_c]):
            pass


def _prep(hidden_states, attention_mask, position_ids, Wq, Wk, Wv, Wo):
    """Host-side sharding/layout. Returns per-core input maps."""
    x = np.asarray(hidden_states, dtype=np.float32)[0]          # [S, HID]
    xT = np.ascontiguousarray(x.T).astype(BF16)                 # [HID, S]

    pos = np.asarray(position_ids)[0].astype(np.float64)        # [S]
    inv = 1.0 / (ROPE_THETA ** (np.arange(0, D, 2, dtype=np.float64) / D))
    ang = np.empty((D, S), dtype=np.float64)
    ang[:64] = inv[:, None] * pos[None, :]
    ang[64:] = ang[:64]
    cosT = np.cos(ang).astype(BF16)
    sinT = np.sin(ang)
    sinT[:64] *= -1.0          # sign folded: halfswap(raw)*sin == rotate_half
    sinT = sinT.astype(BF16)

    m = np.asarray(attention_mask, dtype=np.float32)[0, 0]      # [S, S] additive
    binT = (m > -0.5).astype(np.float32).T                      # [k, q] binary
    blk = binT[0:SB, 0:SB]                                      # diag block
    maskd = np.ascontiguousarray(
        blk.reshape(4, D, SB).transpose(1, 0, 2)).astype(BF16)  # [p, j, q]

    Wq = np.asarray(Wq, dtype=np.float32)
    Wk = np.asarray(Wk, dtype=np.float32)
    Wv = np.asarray(Wv, dtype=np.float32)
    Wo = np.asarray(Wo, dtype=np.float32)

    def pack_kv(W):  # [128, HID] with row p = [W.T[ko*128+p, :] for ko]
        wT = np.ascontiguousarray(W.T)                          # [HID, D]
        return np.ascontiguousarray(
            wT.reshape(NCC, D, D).transpose(1, 0, 2).reshape(D, HID)
        ).astype(BF16)

    in_maps = []
    for c in range(NCORES):
        qsl = slice(c * NQ * D, (c + 1) * NQ * D)
        ksl = slice(c * D, (c + 1) * D)
        in_maps.append({
            "xT": xT,
            "wqT": np.ascontiguousarray(Wq[qsl, :].T).astype(BF16),
            "wkp": pack_kv(Wk[ksl, :]),
            "wvp": pack_kv(Wv[ksl, :]),
            "woT": np.ascontiguousarray(Wo[:, qsl].T).astype(BF16),
            "cosT": cosT, "sinT": sinT, "maskd": maskd,
        })
    return in_maps


def kernel(hidden_states, attention_mask, position_ids, Wq, Wk, Wv, Wo,
           _trace=False):
    from concourse.bass_utils import run_bass_kernel_spmd

    if "nc" not in _CACHE:
        _CACHE["nc"] = _build()
    nc = _CACHE["nc"]

    in_maps = _prep(hidden_states, attention_mask, position_ids, Wq, Wk, Wv, Wo)
    res = run_bass_kernel_spmd(nc, in_maps, core_ids=list(range(NCORES)),
                               trace=_trace)
    _CACHE["last_res"] = res
    out = res.results[0]["part"].astype(np.float64)
    for c in range(1, NCORES):
        out += res.results[c]["part"].astype(np.float64)
    return out.astype(np.float32).reshape(1, S, HID)


if __name__ == "__main__":
    pass
